# revision 26
# baseline (speedup 1.0000x reference)
"""TRN2 Bass kernel for nn_DetProposalRelationHead.

Reference computation (per image b, data-parallel over 8 NeuronCores):
    sub = inst[sub_i]; obj = inst[obj_i]; phr = phrase[phrase_idx]
    h = leaky_relu([sub,obj,phr] @ W1 + b1); logits = h @ W2 + b2
    probs = softmax(logits); p[:,0]=0; pred = max/argmax over classes
    overall = pred_prob * scores[sub_i] * scores[obj_i]; top-100

Key restructuring on device:
  - feat @ W1 = sub@W1a + obj@W1b + phr@W1c.  sub/obj come from only 100
    instance rows, so A = inst@W1a, B = inst@W1b are precomputed ([100,1024])
    and their gathers become one-hot matmuls accumulated in PSUM.
  - Everything runs in "T layout" (hidden dim on partitions, pairs on the
    free dim) so the second matmul contracts over hidden on partitions.
    The gathered phrase rows are transposed on the PE (128x128 tiles).
  - Top-k via HW max8/max_index/match_replace: per-partition top-8
    candidates, then 13 global extraction rounds on a [1,1024] row.

Pair order is partition-major: pair = p*78 + c  (p=partition 0..127,
c=column 0..77; pairs >= 9900 are padding).
"""
import numpy as np

B, NI, D = 8, 100, 1024
P = NI * (NI - 1)          # 9900
C = 51
TOPK = 100
H = 1024
COLS = 78                  # 128*78 = 9984 padded pairs
PADP = 128 * COLS
NIC = 8                    # 1024/128 input chunks
NHC = 8                    # 1024/128 hidden chunks
NB = 20                    # pair-column blocks: 19x4 + 1x2
BSIZES = [4] * 19 + [2]
NROUND = 13                # 13*8 = 104 >= 100 extraction rounds

_cache = {}
_STAGE = [3]  # 1=bulk only, 2=+exact, 3=full (debug knob)


def _build():
    import concourse.bass as bass
    import concourse.mybir as mybir
    from concourse.tile import TileContext

    F32 = mybir.dt.float32
    F32R = mybir.dt.float32r
    I32 = mybir.dt.int32
    U32 = mybir.dt.uint32
    AF = mybir.ActivationFunctionType
    ALU = mybir.AluOpType

    nc = bass.Bass()

    # ---- parameters (per core) ----
    phr = nc.declare_dram_parameter("phr", [P, D], F32R, isOutput=False)
    w1a = nc.declare_dram_parameter("w1a", [D, H], F32, isOutput=False)
    w1b = nc.declare_dram_parameter("w1b", [D, H], F32, isOutput=False)
    w1c = nc.declare_dram_parameter("w1c", [D, H], F32R, isOutput=False)
    instT = nc.declare_dram_parameter("instT", [D, NI], F32, isOutput=False)
    w2 = nc.declare_dram_parameter("w2", [H, C + 1], F32R, isOutput=False)
    b1c = nc.declare_dram_parameter("b1c", [128, NHC], F32, isOutput=False)
    b2r = nc.declare_dram_parameter("b2r", [128, C], F32, isOutput=False)
    scoresc = nc.declare_dram_parameter("scoresc", [NI, 1], F32, isOutput=False)
    subf = nc.declare_dram_parameter("subf", [1, PADP], F32R, isOutput=False)
    objf = nc.declare_dram_parameter("objf", [1, PADP], F32R, isOutput=False)
    idx32 = nc.declare_dram_parameter("idx32", [128, COLS], I32, isOutput=False)
    iota100 = nc.declare_dram_parameter("iota100", [NI, 1], F32R, isOutput=False)
    identf = nc.declare_dram_parameter("identf", [128, 128], F32R, isOutput=False)
    ip78 = nc.declare_dram_parameter("ip78", [128, 1], F32, isOutput=False)
    padmask = nc.declare_dram_parameter("padmask", [128, COLS], F32, isOutput=False)
    cat32 = nc.declare_dram_parameter("cat32", [P, 4], I32, isOutput=False)

    o_logits = nc.declare_dram_parameter("o_logits", [PADP, C], F32, isOutput=True)
    o_probs = nc.declare_dram_parameter("o_probs", [PADP, C], F32, isOutput=True)
    o_tlab = nc.declare_dram_parameter("o_tlab", [1, TOPK], I32, isOutput=True)
    o_tprob = nc.declare_dram_parameter("o_tprob", [1, TOPK], F32, isOutput=True)
    o_tval = nc.declare_dram_parameter("o_tval", [1, TOPK], F32, isOutput=True)
    o_tidx = nc.declare_dram_parameter("o_tidx", [1, TOPK], I32, isOutput=True)

    # ---- internal DRAM scratch ----
    ssp_d = nc.dram_tensor("ssp_d", [PADP, 1], F32)
    candp_d = nc.dram_tensor("candp_d", [128 * 8, 1], F32)
    candl_d = nc.dram_tensor("candl_d", [128 * 8, 1], I32)
    gx_d = nc.dram_tensor("gx_d", [128 * 8, 1], F32)

    with TileContext(nc) as tc:
        with (
            tc.tile_pool(name="persist", bufs=1) as pp,
            tc.tile_pool(name="ss_ps", bufs=1, space="PSUM") as ss_ps,
        ):
            # resident tiles
            w1c_sb = pp.tile([128, NIC, H], F32R)
            nc.sync.dma_start(out=w1c_sb, in_=w1c[:, :].rearrange("(ic p) h -> p ic h", p=128))
            instT_sb = pp.tile([128, NIC, NI], F32)
            nc.sync.dma_start(out=instT_sb, in_=instT[:, :].rearrange("(ic p) n -> p ic n", p=128))
            w2_sb = pp.tile([128, NHC, C + 1], F32R)
            nc.sync.dma_start(out=w2_sb, in_=w2[:, :].rearrange("(hc p) c -> p hc c", p=128))
            w2f_sb = pp.tile([128, NHC, C + 1], F32)
            nc.sync.dma_start(out=w2f_sb, in_=w2[:, :].bitcast(F32).rearrange("(hc p) c -> p hc c", p=128))
            b1c_sb = pp.tile([128, NHC], F32)
            nc.sync.dma_start(out=b1c_sb, in_=b1c[:, :])
            b2r_sb = pp.tile([128, C], F32)
            nc.sync.dma_start(out=b2r_sb, in_=b2r[:, :])
            scores_sb = pp.tile([NI, 1], F32)
            nc.sync.dma_start(out=scores_sb, in_=scoresc[:, :])
            idx_sb = pp.tile([128, COLS], I32)
            nc.sync.dma_start(out=idx_sb, in_=idx32[:, :])
            iota_sb = pp.tile([NI, 1], F32R)
            nc.sync.dma_start(out=iota_sb, in_=iota100[:, :])
            ident_sb = pp.tile([128, 128], F32R)
            nc.sync.dma_start(out=ident_sb, in_=identf[:, :])
            ip78_sb = pp.tile([128, 1], F32)
            nc.sync.dma_start(out=ip78_sb, in_=ip78[:, :])
            pmask_sb = pp.tile([128, COLS], F32)
            nc.sync.dma_start(out=pmask_sb, in_=padmask[:, :])
            ones1f_sb = pp.tile([1, NI], F32)
            nc.vector.memset(ones1f_sb, 1.0)
            ones1_sb = pp.tile([1, NI], F32R)
            nc.vector.tensor_copy(ones1_sb, ones1f_sb)

            A_sb = pp.tile([NI, H], F32)
            B_sb = pp.tile([NI, H], F32)
            A_r = pp.tile([NI, H], F32R)
            B_r = pp.tile([NI, H], F32R)
            ppmat = pp.tile([128, COLS], F32)
            ss_all = ss_ps.tile([128, 2, COLS], F32, space="PSUM")
            ss_sub = ss_all[:, 0, :]
            ss_obj = ss_all[:, 1, :]

            # ---- A/B precompute: A = inst @ W1a, B = inst @ W1b  [100,1024]
            with (
                tc.tile_pool(name="ab_sb", bufs=2) as absb,
                tc.tile_pool(name="ab_ps", bufs=2, space="PSUM") as abps,
            ):
                for M_sb, M_r, W_ in ((A_sb, A_r, w1a), (B_sb, B_r, w1b)):
                    for hb in range(2):
                        ps_ab = abps.tile([NI, 512], F32, space="PSUM", name="ps_ab")
                        for ic in range(NIC):
                            wt = absb.tile([128, 512], F32, name="wt")
                            nc.sync.dma_start(
                                out=wt, in_=W_[ic * 128:(ic + 1) * 128,
                                               hb * 512:(hb + 1) * 512])
                            nc.tensor.matmul(out=ps_ab, lhsT=instT_sb[:, ic, :],
                                             rhs=wt, start=(ic == 0),
                                             stop=(ic == NIC - 1))
                        nc.scalar.activation(M_sb[:, hb * 512:(hb + 1) * 512],
                                             ps_ab, AF.Copy)
                        nc.vector.tensor_copy(M_r[:, hb * 512:(hb + 1) * 512],
                                              ps_ab)

            # ---- main pair-block loop ----
            with (
                tc.tile_pool(name="blk", bufs=2) as bp,
                tc.tile_pool(name="tr_ps", bufs=3, space="PSUM") as trps,
                tc.tile_pool(name="mm1_ps", bufs=2, space="PSUM") as mm1ps,
                tc.tile_pool(name="oh_ps", bufs=1, space="PSUM") as ohps,
                tc.tile_pool(name="mm2_ps", bufs=1, space="PSUM") as mm2ps,
            ):
                for bi in range(NB):
                    bs = BSIZES[bi]
                    c0 = 4 * bi
                    nb = bs * 128

                    # gather phrase rows for this block's pairs
                    phrg = bp.tile([128, 4, D], F32R, name="phrg")
                    for j in range(bs):
                        nc.gpsimd.indirect_dma_start(
                            out=phrg[:, j, :], out_offset=None, in_=phr[:, :],
                            in_offset=bass.IndirectOffsetOnAxis(
                                ap=idx_sb[:, c0 + j:c0 + j + 1], axis=0))

                    # PE transpose to T layout [in, pairs]
                    phrT = bp.tile([128, NIC, 4, 128], F32R, name="phrT")
                    for ic in range(NIC):
                        for j in range(bs):
                            pst = trps.tile([128, 128], F32R, space="PSUM", name="pst")
                            nc.tensor.transpose(
                                pst, phrg[:, j, ic * 128:(ic + 1) * 128], ident_sb)
                            eng = nc.scalar if (ic + j) % 2 == 0 else nc.vector
                            if eng is nc.scalar:
                                nc.scalar.activation(phrT[:, ic, j, :], pst, AF.Copy)
                            else:
                                nc.vector.tensor_copy(phrT[:, ic, j, :], pst)

                    # one-hot matrices for sub/obj of this block
                    ohs = bp.tile([NI, 4, 128], F32R, name="ohs")
                    oho = bp.tile([NI, 4, 128], F32R, name="oho")
                    ohsf = bp.tile([NI, 4, 128], F32, name="ohsf")
                    ohof = bp.tile([NI, 4, 128], F32, name="ohof")
                    for oh, ohf, srcd in ((ohs, ohsf, subf), (oho, ohof, objf)):
                        srow = bp.tile([1, 512], F32R, name="srow")
                        nc.sync.dma_start(out=srow[:, :nb],
                                          in_=srcd[0:1, c0 * 128:(c0 + bs) * 128])
                        ps_oh = ohps.tile([NI, 4, 128], F32, space="PSUM", name="ps_oh")
                        nc.tensor.matmul(
                            out=ps_oh[:, :bs, :].rearrange("p b x -> p (b x)"),
                            lhsT=ones1_sb,
                            rhs=srow[0:1, :nb],
                            start=True, stop=True)
                        rep = bp.tile([NI, 4, 128], F32, name="rep")
                        nc.scalar.activation(rep[:, :bs, :], ps_oh[:, :bs, :], AF.Copy)
                        nc.vector.tensor_tensor(
                            out=oh[:, :bs, :],
                            in0=iota_sb.to_broadcast([NI, bs, 128]),
                            in1=rep[:, :bs, :], op=ALU.is_equal)
                        nc.vector.tensor_tensor(
                            out=ohf[:, :bs, :],
                            in0=iota_sb.bitcast(F32).to_broadcast([NI, bs, 128]),
                            in1=rep[:, :bs, :], op=ALU.is_equal)

                    # mm1: pre_h_T accumulation + bias + leaky relu
                    hT = bp.tile([128, NHC, 4, 128], F32R, name="hT")
                    for hc in range(NHC):
                        ps_h = mm1ps.tile([128, 512], F32, space="PSUM", name="ps_h")
                        for ic in range(NIC):
                            nc.tensor.matmul(
                                out=ps_h[:, :nb],
                                lhsT=w1c_sb[:, ic, hc * 128:(hc + 1) * 128],
                                rhs=phrT[:, ic, :bs, :].rearrange("p b x -> p (b x)"),
                                start=(ic == 0), stop=False)
                        nc.tensor.matmul(
                            out=ps_h[:, :nb],
                            lhsT=A_r[:, hc * 128:(hc + 1) * 128],
                            rhs=ohs[:, :bs, :].rearrange("p b x -> p (b x)"),
                            start=False, stop=False)
                        nc.tensor.matmul(
                            out=ps_h[:, :nb],
                            lhsT=B_r[:, hc * 128:(hc + 1) * 128],
                            rhs=oho[:, :bs, :].rearrange("p b x -> p (b x)"),
                            start=False, stop=True)
                        nc.scalar.activation(
                            hT[:, hc, :bs, :].rearrange("p b x -> p (b x)"),
                            ps_h[:, :nb], AF.Lrelu,
                            bias=b1c_sb[:, hc:hc + 1], alpha=0.01)

                    # mm2: logits per column chunk
                    ps_lg = mm2ps.tile([128, 4, C + 1], F32, space="PSUM", name="ps_lg")
                    for j in range(bs):
                        for hc in range(NHC):
                            nc.tensor.matmul(out=ps_lg[:, j, :],
                                             lhsT=hT[:, hc, j, :],
                                             rhs=w2_sb[:, hc, :],
                                             start=(hc == 0), stop=(hc == NHC - 1))
                    lg = bp.tile([128, 4, C], F32, name="lg")
                    nc.vector.tensor_tensor(
                        out=lg[:, :bs, :], in0=ps_lg[:, :bs, :C],
                        in1=b2r_sb.unsqueeze(1).to_broadcast([128, bs, C]),
                        op=ALU.add)
                    nc.sync.dma_start(
                        out=o_logits[:, :].rearrange("(c p) k -> p c k", p=128)[:, c0:c0 + bs, :],
                        in_=lg[:, :bs, :])

                    # softmax (no max-subtraction; logits are O(1))
                    e = bp.tile([128, 4, C], F32, name="e")
                    se = bp.tile([128, 4], F32, name="se")
                    for j in range(bs):
                        nc.scalar.activation(e[:, j, :], lg[:, j, :], AF.Exp,
                                             accum_out=se[:, j:j + 1])
                    re = bp.tile([128, 4], F32, name="re")
                    nc.vector.reciprocal(re[:, :bs], se[:, :bs])
                    pb = bp.tile([128, 4, C], F32, name="pb")
                    for j in range(bs):
                        nc.scalar.activation(pb[:, j, :], e[:, j, :], AF.Copy,
                                             scale=re[:, j:j + 1])
                    nc.sync.dma_start(
                        out=o_probs[:, :].rearrange("(c p) k -> p c k", p=128)[:, c0:c0 + bs, :],
                        in_=pb[:, :bs, :])

                    # per-pair class max (class 0 excluded) + ss matmuls
                    nc.vector.tensor_reduce(ppmat[:, c0:c0 + bs],
                                            pb[:, :bs, 1:C],
                                            axis=mybir.AxisListType.X,
                                            op=ALU.max)
                    for j in range(bs):
                        nc.tensor.matmul(out=ss_sub[:, c0 + j:c0 + j + 1],
                                         lhsT=ohsf[:, j, :], rhs=scores_sb,
                                         start=True, stop=True)
                        nc.tensor.matmul(out=ss_obj[:, c0 + j:c0 + j + 1],
                                         lhsT=ohof[:, j, :], rhs=scores_sb,
                                         start=True, stop=True)

            if _STAGE[0] < 1.5:
                return nc
            # ---- overall score + padding mask ----
            with tc.tile_pool(name="topk_sb", bufs=1) as tk:
                sssb = tk.tile([128, COLS], F32)
                nc.scalar.activation(sssb, ss_sub, AF.Copy)
                ssp = tk.tile([128, COLS], F32)
                nc.vector.tensor_tensor(out=ssp, in0=sssb, in1=ss_obj,
                                        op=ALU.mult)
                nc.sync.dma_start(
                    out=ssp_d[:, :].rearrange("(c p) o -> p (c o)", p=128),
                    in_=ssp)
                ov0 = tk.tile([128, COLS], F32)
                nc.vector.tensor_tensor(out=ov0, in0=ppmat, in1=ssp, op=ALU.mult)
                overall = tk.tile([128, COLS], F32)
                nc.vector.tensor_tensor(out=overall, in0=ov0, in1=pmask_sb,
                                        op=ALU.min)

                # candidates: per-partition top-8 + global pair index
                m8 = tk.tile([128, 8], F32)
                i8 = tk.tile([128, 8], U32)
                nc.vector.max(out=m8, in_=overall)
                nc.vector.max_index(out=i8, in_max=m8, in_values=overall)
                gfa = tk.tile([128, 8], F32)
                nc.vector.tensor_copy(gfa, i8)
                gf = tk.tile([128, 8], F32)
                nc.vector.tensor_scalar(gf, gfa, 128.0, ip78_sb[:, 0:1],
                                        op0=ALU.mult, op1=ALU.add)
                gint = tk.tile([128, 8], I32)
                nc.vector.tensor_copy(gint, gf)

                # --- exact fp32 recompute + re-ranking of the 1024 candidates ---
                css = tk.tile([128, 8], F32)
                cmeta = tk.tile([128, 8, 4], I32)
                for e in range(8):
                    nc.gpsimd.indirect_dma_start(
                        out=css[:, e:e + 1], out_offset=None, in_=ssp_d[:, :],
                        in_offset=bass.IndirectOffsetOnAxis(
                            ap=gint[:, e:e + 1], axis=0))
                    nc.gpsimd.indirect_dma_start(
                        out=cmeta[:, e, :], out_offset=None, in_=cat32[:, :],
                        in_offset=bass.IndirectOffsetOnAxis(
                            ap=gint[:, e:e + 1], axis=0))
                cpi = tk.tile([128, 8], I32)
                nc.vector.tensor_copy(cpi, cmeta[:, :, 0])

                # sub/obj values of candidates as flat [1,1024] rows
                csubF = tk.tile([128, 8], F32)
                nc.vector.tensor_copy(csubF, cmeta[:, :, 1])
                cobjF = tk.tile([128, 8], F32)
                nc.vector.tensor_copy(cobjF, cmeta[:, :, 2])
                # transpose to [8,128] so the flat row is in e*128+p order
                with tc.tile_pool(name="ctr_ps", bufs=1, space="PSUM") as ctrp:
                    csub_row = tk.tile([1, 1024], F32)
                    cobj_row = tk.tile([1, 1024], F32)
                    for colF, row in ((csubF, csub_row), (cobjF, cobj_row)):
                        psct = ctrp.tile([8, 128], F32, space="PSUM", name="psct")
                        nc.tensor.transpose(psct, colF, ident_sb.bitcast(F32))
                        ctT = tk.tile([8, 128], F32, name="ctT", tag="ctT")
                        nc.vector.tensor_copy(ctT, psct)
                        nc.sync.dma_start(
                            out=row.rearrange("o (q r) -> o q r", q=8), in_=ctT)

                # fp32 W1c reload (raw bytes view of the f32r param)
                w1cf_sb = tk.tile([128, NIC, H], F32)
                nc.sync.dma_start(
                    out=w1cf_sb,
                    in_=w1c[:, :].bitcast(F32).rearrange("(ic p) h -> p ic h", p=128))

                clg = tk.tile([128, 8, C], F32)
                with (
                    tc.tile_pool(name="ex_sb", bufs=1) as xb,
                    tc.tile_pool(name="ex_tr", bufs=2, space="PSUM") as xtr,
                    tc.tile_pool(name="ex_mm1", bufs=2, space="PSUM") as xmm1,
                    tc.tile_pool(name="ex_oh", bufs=1, space="PSUM") as xoh,
                    tc.tile_pool(name="ex_mm2", bufs=1, space="PSUM") as xmm2,
                ):
                    for cb in range(2):
                        er = range(4 * cb, 4 * cb + 4)
                        # gather candidate phrase rows (exact fp32 bytes)
                        phrgc = xb.tile([128, 4, D], F32, name="phrgc", bufs=2)
                        for k, e in enumerate(er):
                            nc.gpsimd.indirect_dma_start(
                                out=phrgc[:, k, :], out_offset=None,
                                in_=phr[:, :].bitcast(F32),
                                in_offset=bass.IndirectOffsetOnAxis(
                                    ap=cpi[:, e:e + 1], axis=0))
                        phrTc = xb.tile([128, NIC, 4, 128], F32, name="phrTc")
                        for ic in range(NIC):
                            for k in range(4):
                                pstx = xtr.tile([128, 128], F32, space="PSUM",
                                                name="pstx")
                                nc.tensor.transpose(
                                    pstx, phrgc[:, k, ic * 128:(ic + 1) * 128],
                                    ident_sb.bitcast(F32))
                                if (ic + k) % 2 == 0:
                                    nc.scalar.activation(phrTc[:, ic, k, :], pstx,
                                                         AF.Copy)
                                else:
                                    nc.vector.tensor_copy(phrTc[:, ic, k, :], pstx)
                        # candidate one-hots (fp32)
                        ohcs = xb.tile([NI, 4, 128], F32, name="ohcs")
                        ohco = xb.tile([NI, 4, 128], F32, name="ohco")
                        for ohc, rowt in ((ohcs, csub_row), (ohco, cobj_row)):
                            ps_ohc = xoh.tile([NI, 4, 128], F32, space="PSUM",
                                              name="ps_ohc")
                            nc.tensor.matmul(
                                out=ps_ohc.rearrange("p b x -> p (b x)"),
                                lhsT=ones1f_sb,
                                rhs=rowt[0:1, 512 * cb:512 * (cb + 1)],
                                start=True, stop=True)
                            repc = xb.tile([NI, 4, 128], F32, name="repc")
                            nc.scalar.activation(repc, ps_ohc, AF.Copy)
                            nc.vector.tensor_tensor(
                                out=ohc,
                                in0=iota_sb.bitcast(F32).to_broadcast([NI, 4, 128]),
                                in1=repc, op=ALU.is_equal)
                        # exact mm1 + leaky
                        hTc = xb.tile([128, NHC, 4, 128], F32, name="hTc")
                        for hc in range(NHC):
                            psx = xmm1.tile([128, 512], F32, space="PSUM", name="psx")
                            for ic in range(NIC):
                                nc.tensor.matmul(
                                    out=psx,
                                    lhsT=w1cf_sb[:, ic, hc * 128:(hc + 1) * 128],
                                    rhs=phrTc[:, ic, :, :].rearrange("p b x -> p (b x)"),
                                    start=(ic == 0), stop=False)
                            nc.tensor.matmul(
                                out=psx, lhsT=A_sb[:, hc * 128:(hc + 1) * 128],
                                rhs=ohcs.rearrange("p b x -> p (b x)"),
                                start=False, stop=False)
                            nc.tensor.matmul(
                                out=psx, lhsT=B_sb[:, hc * 128:(hc + 1) * 128],
                                rhs=ohco.rearrange("p b x -> p (b x)"),
                                start=False, stop=True)
                            nc.scalar.activation(
                                hTc[:, hc, :, :].rearrange("p b x -> p (b x)"),
                                psx, AF.Lrelu, bias=b1c_sb[:, hc:hc + 1],
                                alpha=0.01)
                        # exact mm2 + b2
                        ps_lx = xmm2.tile([128, 4, C], F32, space="PSUM", name="ps_lx")
                        for k in range(4):
                            for hc in range(NHC):
                                nc.tensor.matmul(out=ps_lx[:, k, :],
                                                 lhsT=hTc[:, hc, k, :],
                                                 rhs=w2f_sb[:, hc, :C],
                                                 start=(hc == 0),
                                                 stop=(hc == NHC - 1))
                        nc.vector.tensor_tensor(
                            out=clg[:, 4 * cb:4 * cb + 4, :], in0=ps_lx,
                            in1=b2r_sb.unsqueeze(1).to_broadcast([128, 4, C]),
                            op=ALU.add)

                # stable softmax pieces with polynomial exp (~1e-7 rel)
                lmax = tk.tile([128, 8], F32)
                nc.vector.tensor_reduce(lmax, clg, axis=mybir.AxisListType.X,
                                        op=ALU.max)
                x = tk.tile([128, 8, C], F32)
                nc.vector.tensor_tensor(
                    out=x, in0=clg,
                    in1=lmax.unsqueeze(2).to_broadcast([128, 8, C]),
                    op=ALU.subtract)
                LOG2E, LN2 = 1.4426950408889634, 0.6931471805599453
                t = tk.tile([128, 8, C], F32)
                nc.vector.tensor_scalar(t, x, LOG2E, None, op0=ALU.mult)
                ki = tk.tile([128, 8, C], I32)
                nc.vector.tensor_copy(ki, t)
                kf = tk.tile([128, 8, C], F32)
                nc.vector.tensor_copy(kf, ki)
                u = tk.tile([128, 8, C], F32)
                nc.vector.tensor_scalar(u, kf, LN2, None, op0=ALU.mult)
                r = tk.tile([128, 8, C], F32)
                nc.vector.tensor_tensor(out=r, in0=x, in1=u, op=ALU.subtract)
                ke0 = tk.tile([128, 8, C], I32)
                nc.vector.tensor_scalar(ke0, ki, 127, None, op0=ALU.add)
                ke = tk.tile([128, 8, C], I32)
                nc.vector.tensor_scalar(ke, ke0, 23, None,
                                        op0=ALU.arith_shift_left)
                # Horner for e^r, r in (-ln2, ln2)
                import math
                coef = [1.0 / math.factorial(n) for n in range(10)]
                pol = tk.tile([128, 8, C], F32)
                nc.vector.tensor_scalar(pol, r, coef[9], coef[8],
                                        op0=ALU.mult, op1=ALU.add)
                for n in range(7, -1, -1):
                    pm = tk.tile([128, 8, C], F32, name=f"pm{n}", tag="pm")
                    nc.vector.tensor_tensor(out=pm, in0=pol, in1=r, op=ALU.mult)
                    pol = tk.tile([128, 8, C], F32, name=f"po{n}", tag="po")
                    nc.vector.tensor_scalar(pol, pm, coef[n], None, op0=ALU.add)
                eall = tk.tile([128, 8, C], F32)
                nc.vector.tensor_tensor(out=eall, in0=pol,
                                        in1=ke.bitcast(F32), op=ALU.mult)
                sexp = tk.tile([128, 8], F32)
                nc.vector.tensor_reduce(sexp, eall, axis=mybir.AxisListType.X,
                                        op=ALU.add)
                # reciprocal + one Newton step
                r0 = tk.tile([128, 8], F32)
                nc.vector.reciprocal(r0, sexp)
                t2 = tk.tile([128, 8], F32)
                nc.vector.tensor_tensor(out=t2, in0=sexp, in1=r0, op=ALU.mult)
                u2 = tk.tile([128, 8], F32)
                nc.vector.tensor_scalar(u2, t2, -1.0, 2.0,
                                        op0=ALU.mult, op1=ALU.add)
                r1 = tk.tile([128, 8], F32)
                nc.vector.tensor_tensor(out=r1, in0=r0, in1=u2, op=ALU.mult)
                # class-0-excluded max + argmax
                emax1 = tk.tile([128, 8], F32)
                labs = tk.tile([128, 8], I32)
                for e in range(8):
                    cm8 = tk.tile([128, 8], F32, name=f"cm8_{e}", tag="cm8")
                    ci8 = tk.tile([128, 8], U32, name=f"ci8_{e}", tag="ci8")
                    nc.vector.max(out=cm8, in_=eall[:, e, 1:C])
                    nc.vector.max_index(out=ci8, in_max=cm8,
                                        in_values=eall[:, e, 1:C])
                    nc.vector.tensor_copy(emax1[:, e:e + 1], cm8[:, 0:1])
                    nc.vector.tensor_scalar(labs[:, e:e + 1], ci8[:, 0:1], 1,
                                            None, op0=ALU.add)
                pred = tk.tile([128, 8], F32)
                nc.vector.tensor_tensor(out=pred, in0=emax1, in1=r1, op=ALU.mult)
                novr = tk.tile([128, 8], F32)
                nc.vector.tensor_tensor(out=novr, in0=pred, in1=css, op=ALU.mult)

                # stash candidate pred/label by candidate position j=p*8+e
                nc.sync.dma_start(
                    out=candp_d[:, :].rearrange("(p e) o -> p (e o)", e=8),
                    in_=pred)
                nc.sync.dma_start(
                    out=candl_d[:, :].rearrange("(p e) o -> p (e o)", e=8),
                    in_=labs)

                if _STAGE[0] < 2.5:
                    return nc
                rv = tk.tile([1, 1024], F32)
                nc.sync.dma_start(out=rv.rearrange("o (p e) -> o p e", p=128),
                                  in_=novr)
                rg = tk.tile([1, 1024], F32)
                nc.sync.dma_start(out=rg.rearrange("o (p e) -> o p e", p=128), in_=gf)
                nc.sync.dma_start(out=gx_d[:, :].rearrange("p o -> o p"), in_=rg)

                # 13 global extraction rounds -> top-104 sorted
                tvals = tk.tile([1, NROUND * 8], F32)
                tpos = tk.tile([1, NROUND * 8], F32)
                for k in range(NROUND):
                    rm8 = tk.tile([1, 8], F32, name="rm8", tag="rm8")
                    ri8 = tk.tile([1, 8], U32, name="ri8", tag="ri8")
                    nc.vector.max(out=rm8, in_=rv)
                    nc.vector.max_index(out=ri8, in_max=rm8, in_values=rv)
                    nc.vector.match_replace(out=rv, in_to_replace=rm8,
                                            in_values=rv, imm_value=-1e30)
                    nc.vector.tensor_copy(tvals[:, 8 * k:8 * k + 8], rm8)
                    nc.vector.tensor_copy(tpos[:, 8 * k:8 * k + 8], ri8)

                # positions -> column vector -> int
                tposc = tk.tile([NROUND * 8, 1], F32)
                nc.sync.dma_start(
                    out=tposc, in_=tpos.rearrange("o (p e) -> o p e", p=NROUND * 8))
                tposi = tk.tile([NROUND * 8, 1], I32)
                nc.vector.tensor_copy(tposi, tposc)

                # gather winner pair idx / pred prob / label by position
                gxc = tk.tile([NROUND * 8, 1], F32)
                nc.gpsimd.indirect_dma_start(
                    out=gxc, out_offset=None, in_=gx_d[:, :],
                    in_offset=bass.IndirectOffsetOnAxis(ap=tposi[:, :1], axis=0))
                gxi = tk.tile([NROUND * 8, 1], I32)
                nc.vector.tensor_copy(gxi, gxc)
                tprobc = tk.tile([NROUND * 8, 1], F32)
                nc.gpsimd.indirect_dma_start(
                    out=tprobc, out_offset=None, in_=candp_d[:, :],
                    in_offset=bass.IndirectOffsetOnAxis(ap=tposi[:, :1], axis=0))
                tlabc = tk.tile([NROUND * 8, 1], I32)
                nc.gpsimd.indirect_dma_start(
                    out=tlabc, out_offset=None, in_=candl_d[:, :],
                    in_offset=bass.IndirectOffsetOnAxis(ap=tposi[:, :1], axis=0))

                # outputs
                nc.sync.dma_start(out=o_tval[:, :], in_=tvals[:, :TOPK])
                nc.sync.dma_start(
                    out=o_tidx[:, :].rearrange("o (p e) -> o p e", p=TOPK),
                    in_=gxi[:TOPK, :])
                nc.sync.dma_start(
                    out=o_tlab[:, :].rearrange("o (p e) -> o p e", p=TOPK),
                    in_=tlabc[:TOPK, :])
                nc.sync.dma_start(
                    out=o_tprob[:, :].rearrange("o (p e) -> o p e", p=TOPK),
                    in_=tprobc[:TOPK, :])

    _fix_multiwait(nc)
    return nc


def _fix_multiwait(nc, keep=1):
    """walrus (this snapshot) rejects instructions whose sync_info carries
    more waits than the ISA CTRL struct holds (the Tile tail Drain gets one
    wait per outstanding semaphore). Spill excess waits onto standalone NOPs
    inserted immediately before the instruction — NX executes them in program
    order, so semantics are identical."""
    import concourse.mybir as mybir
    ctr = [0]
    for f in nc.m.functions:
        for bb in f.blocks:
            insts = bb.instructions
            i = 0
            while i < len(insts):
                ins = insts[i]
                si = ins.sync_info
                if si is not None and len(si.on_wait) > keep:
                    waits = list(si.on_wait)
                    spill, rest = waits[:-keep], waits[-keep:]
                    for w in spill:
                        ctr[0] += 1
                        nop = mybir.InstNoOp(
                            name=f"I-waitspill-{ctr[0]}", ins=[], outs=[])
                        nop.engine = ins.engine
                        nop.sync_info = mybir.SyncInfo(on_wait=[w], on_update=[])
                        insts.insert(i, nop)
                        i += 1
                    ins.sync_info = mybir.SyncInfo(
                        on_wait=rest, on_update=list(si.on_update))
                i += 1


def _host_prep(inputs):
    """Per-core input maps. All host work is layout/marshalling only."""
    inst = np.asarray(inputs["instance_features"], np.float32)   # [B,100,1024]
    phr = np.asarray(inputs["phrase_features"], np.float32)      # [B,9900,1024]
    scores = np.asarray(inputs["scores"], np.float32)            # [B,100]
    W1 = np.asarray(inputs["W1"], np.float32)                    # [3072,1024]
    b1 = np.asarray(inputs["b1"], np.float32)
    W2 = np.asarray(inputs["W2"], np.float32)
    b2 = np.asarray(inputs["b2"], np.float32)
    conn = np.asarray(inputs["connect_arr"])                     # [B,2,9900]
    pidx = np.asarray(inputs["phrase_idx"])                      # [B,9900]

    w2p = np.concatenate([W2, np.zeros((H, 1), np.float32)], axis=1)
    w1a = np.ascontiguousarray(W1[:D])
    w1b = np.ascontiguousarray(W1[D:2 * D])
    w1c = np.ascontiguousarray(W1[2 * D:])
    b1c = np.ascontiguousarray(b1.reshape(NHC, 128).T)           # [128,8]
    b2r = np.ascontiguousarray(np.broadcast_to(b2, (128, C)))
    iota = np.arange(NI, dtype=np.float32).reshape(NI, 1)
    ident = np.eye(128, dtype=np.float32)
    ip78 = np.arange(128, dtype=np.float32).reshape(128, 1)
    pmask = np.ascontiguousarray(
        np.where(np.arange(PADP) < P, 1e30, -1e30).astype(np.float32)
        .reshape(COLS, 128).T)

    def pad_pair(a, fill=0):
        out = np.full(PADP, fill, a.dtype)
        out[:P] = a
        return out

    maps = []
    for b in range(B):
        sub_i = conn[b, 0].astype(np.int64)
        obj_i = conn[b, 1].astype(np.int64)
        pi = pidx[b].astype(np.int64)
        # partition-major order: pair = p*78 + c  ->  array[p, c]
        subf = pad_pair(sub_i.astype(np.float32)).reshape(1, PADP)
        objf = pad_pair(obj_i.astype(np.float32)).reshape(1, PADP)
        idx32 = np.ascontiguousarray(pad_pair(pi.astype(np.int32)).reshape(COLS, 128).T)
        maps.append({
            "phr": np.ascontiguousarray(phr[b]),
            "w1a": w1a, "w1b": w1b, "w1c": w1c,
            "instT": np.ascontiguousarray(inst[b].T),
            "w2": w2p, "b1c": b1c, "b2r": b2r,
            "scoresc": np.ascontiguousarray(scores[b].reshape(NI, 1)),
            "subf": np.ascontiguousarray(subf),
            "objf": np.ascontiguousarray(objf),
            "idx32": idx32,
            "iota100": iota, "identf": ident, "ip78": ip78,
            "padmask": pmask,
            "cat32": np.ascontiguousarray(np.stack(
                [pi, sub_i, obj_i, np.zeros(P, np.int64)], axis=1).astype(np.int32)),
        })
    return maps


def kernel(**inputs):
    from concourse.bass_utils import run_bass_kernel_spmd

    if "nc" not in _cache:
        _cache["nc"] = _build()
    nc = _cache["nc"]

    maps = _host_prep(inputs)
    res = run_bass_kernel_spmd(nc, maps, list(range(B))).results

    logits = np.stack([res[b]["o_logits"][:P] for b in range(B)])
    probs = np.stack([res[b]["o_probs"][:P] for b in range(B)])
    tlab = np.stack([res[b]["o_tlab"][0] for b in range(B)]).astype(np.int32)
    tprob = np.stack([res[b]["o_tprob"][0] for b in range(B)])
    tval = np.stack([res[b]["o_tval"][0] for b in range(B)])
    tidx = np.stack([res[b]["o_tidx"][0] for b in range(B)]).astype(np.int32)
    return logits, probs, tlab, tprob, tval, tidx


# revision 29
# speedup vs baseline: 1.0080x; 1.0080x over previous
"""TRN2 Bass kernel for nn_DetProposalRelationHead.

Reference computation (per image b, data-parallel over 8 NeuronCores):
    sub = inst[sub_i]; obj = inst[obj_i]; phr = phrase[phrase_idx]
    h = leaky_relu([sub,obj,phr] @ W1 + b1); logits = h @ W2 + b2
    probs = softmax(logits); p[:,0]=0; pred = max/argmax over classes
    overall = pred_prob * scores[sub_i] * scores[obj_i]; top-100

Key restructuring on device:
  - feat @ W1 = sub@W1a + obj@W1b + phr@W1c.  sub/obj come from only 100
    instance rows, so A = inst@W1a, B = inst@W1b are precomputed ([100,1024])
    and their gathers become one-hot matmuls accumulated in PSUM.
  - Everything runs in "T layout" (hidden dim on partitions, pairs on the
    free dim) so the second matmul contracts over hidden on partitions.
    The gathered phrase rows are transposed on the PE (128x128 tiles).
  - Top-k via HW max8/max_index/match_replace: per-partition top-8
    candidates, then 13 global extraction rounds on a [1,1024] row.

Pair order is partition-major: pair = p*78 + c  (p=partition 0..127,
c=column 0..77; pairs >= 9900 are padding).
"""
import numpy as np

B, NI, D = 8, 100, 1024
P = NI * (NI - 1)          # 9900
C = 51
TOPK = 100
H = 1024
COLS = 78                  # 128*78 = 9984 padded pairs
PADP = 128 * COLS
NIC = 8                    # 1024/128 input chunks
NHC = 8                    # 1024/128 hidden chunks
NB = 20                    # pair-column blocks: 19x4 + 1x2
BSIZES = [4] * 19 + [2]
NROUND = 13                # 13*8 = 104 >= 100 extraction rounds

_cache = {}
_STAGE = [3]  # 1=bulk only, 2=+exact, 3=full (debug knob)


def _build():
    import concourse.bass as bass
    import concourse.mybir as mybir
    from concourse.tile import TileContext

    F32 = mybir.dt.float32
    F32R = mybir.dt.float32r
    I32 = mybir.dt.int32
    U32 = mybir.dt.uint32
    AF = mybir.ActivationFunctionType
    ALU = mybir.AluOpType

    nc = bass.Bass()

    # ---- parameters (per core) ----
    phr = nc.declare_dram_parameter("phr", [P, D], F32R, isOutput=False)
    w1a = nc.declare_dram_parameter("w1a", [D, H], F32, isOutput=False)
    w1b = nc.declare_dram_parameter("w1b", [D, H], F32, isOutput=False)
    w1c = nc.declare_dram_parameter("w1c", [D, H], F32R, isOutput=False)
    instT = nc.declare_dram_parameter("instT", [D, NI], F32, isOutput=False)
    w2 = nc.declare_dram_parameter("w2", [H, C + 1], F32R, isOutput=False)
    b1c = nc.declare_dram_parameter("b1c", [128, NHC], F32, isOutput=False)
    b2r = nc.declare_dram_parameter("b2r", [128, C], F32, isOutput=False)
    scoresc = nc.declare_dram_parameter("scoresc", [NI, 1], F32, isOutput=False)
    subf = nc.declare_dram_parameter("subf", [1, PADP], F32R, isOutput=False)
    objf = nc.declare_dram_parameter("objf", [1, PADP], F32R, isOutput=False)
    idx32 = nc.declare_dram_parameter("idx32", [128, COLS], I32, isOutput=False)
    iota100 = nc.declare_dram_parameter("iota100", [NI, 1], F32R, isOutput=False)
    identf = nc.declare_dram_parameter("identf", [128, 128], F32R, isOutput=False)
    ip78 = nc.declare_dram_parameter("ip78", [128, 1], F32, isOutput=False)
    padmask = nc.declare_dram_parameter("padmask", [128, COLS], F32, isOutput=False)
    cat32 = nc.declare_dram_parameter("cat32", [P, 4], I32, isOutput=False)

    o_logits = nc.declare_dram_parameter("o_logits", [PADP, C], F32, isOutput=True)
    o_probs = nc.declare_dram_parameter("o_probs", [PADP, C], F32, isOutput=True)
    o_tlab = nc.declare_dram_parameter("o_tlab", [1, TOPK], I32, isOutput=True)
    o_tprob = nc.declare_dram_parameter("o_tprob", [1, TOPK], F32, isOutput=True)
    o_tval = nc.declare_dram_parameter("o_tval", [1, TOPK], F32, isOutput=True)
    o_tidx = nc.declare_dram_parameter("o_tidx", [1, TOPK], I32, isOutput=True)

    # ---- internal DRAM scratch ----
    ssp_d = nc.dram_tensor("ssp_d", [PADP, 1], F32)
    candp_d = nc.dram_tensor("candp_d", [128 * 8, 1], F32)
    candl_d = nc.dram_tensor("candl_d", [128 * 8, 1], I32)
    gx_d = nc.dram_tensor("gx_d", [128 * 8, 1], F32)

    with TileContext(nc) as tc:
        with (
            tc.tile_pool(name="persist", bufs=1) as pp,
            tc.tile_pool(name="ss_ps", bufs=1, space="PSUM") as ss_ps,
        ):
            # resident tiles
            w1c_sb = pp.tile([128, NIC, H], F32R)
            nc.sync.dma_start(out=w1c_sb, in_=w1c[:, :].rearrange("(ic p) h -> p ic h", p=128))
            instT_sb = pp.tile([128, NIC, NI], F32)
            nc.sync.dma_start(out=instT_sb, in_=instT[:, :].rearrange("(ic p) n -> p ic n", p=128))
            w2_sb = pp.tile([128, NHC, C + 1], F32R)
            nc.sync.dma_start(out=w2_sb, in_=w2[:, :].rearrange("(hc p) c -> p hc c", p=128))
            w2f_sb = pp.tile([128, NHC, C + 1], F32)
            nc.sync.dma_start(out=w2f_sb, in_=w2[:, :].bitcast(F32).rearrange("(hc p) c -> p hc c", p=128))
            b1c_sb = pp.tile([128, NHC], F32)
            nc.sync.dma_start(out=b1c_sb, in_=b1c[:, :])
            b2r_sb = pp.tile([128, C], F32)
            nc.sync.dma_start(out=b2r_sb, in_=b2r[:, :])
            scores_sb = pp.tile([NI, 1], F32)
            nc.sync.dma_start(out=scores_sb, in_=scoresc[:, :])
            idx_sb = pp.tile([128, COLS], I32)
            nc.sync.dma_start(out=idx_sb, in_=idx32[:, :])
            iota_sb = pp.tile([NI, 1], F32R)
            nc.sync.dma_start(out=iota_sb, in_=iota100[:, :])
            ident_sb = pp.tile([128, 128], F32R)
            nc.sync.dma_start(out=ident_sb, in_=identf[:, :])
            ip78_sb = pp.tile([128, 1], F32)
            nc.sync.dma_start(out=ip78_sb, in_=ip78[:, :])
            pmask_sb = pp.tile([128, COLS], F32)
            nc.sync.dma_start(out=pmask_sb, in_=padmask[:, :])
            ones1f_sb = pp.tile([1, NI], F32)
            nc.vector.memset(ones1f_sb, 1.0)
            ones1_sb = pp.tile([1, NI], F32R)
            nc.vector.tensor_copy(ones1_sb, ones1f_sb)

            A_sb = pp.tile([NI, H], F32)
            B_sb = pp.tile([NI, H], F32)
            A_r = pp.tile([NI, H], F32R)
            B_r = pp.tile([NI, H], F32R)
            ppmat = pp.tile([128, COLS], F32)
            ss_all = ss_ps.tile([128, 2, COLS], F32, space="PSUM")
            ss_sub = ss_all[:, 0, :]
            ss_obj = ss_all[:, 1, :]

            # ---- A/B precompute: A = inst @ W1a, B = inst @ W1b  [100,1024]
            with (
                tc.tile_pool(name="ab_sb", bufs=2) as absb,
                tc.tile_pool(name="ab_ps", bufs=2, space="PSUM") as abps,
            ):
                for M_sb, M_r, W_ in ((A_sb, A_r, w1a), (B_sb, B_r, w1b)):
                    for hb in range(2):
                        ps_ab = abps.tile([NI, 512], F32, space="PSUM", name="ps_ab")
                        for ic in range(NIC):
                            wt = absb.tile([128, 512], F32, name="wt")
                            nc.sync.dma_start(
                                out=wt, in_=W_[ic * 128:(ic + 1) * 128,
                                               hb * 512:(hb + 1) * 512])
                            nc.tensor.matmul(out=ps_ab, lhsT=instT_sb[:, ic, :],
                                             rhs=wt, start=(ic == 0),
                                             stop=(ic == NIC - 1))
                        nc.scalar.activation(M_sb[:, hb * 512:(hb + 1) * 512],
                                             ps_ab, AF.Copy)
                        nc.vector.tensor_copy(M_r[:, hb * 512:(hb + 1) * 512],
                                              ps_ab)

            # ---- main pair-block loop ----
            with (
                tc.tile_pool(name="blk", bufs=2) as bp,
                tc.tile_pool(name="tr_ps", bufs=3, space="PSUM") as trps,
                tc.tile_pool(name="mm1_ps", bufs=2, space="PSUM") as mm1ps,
                tc.tile_pool(name="oh_ps", bufs=1, space="PSUM") as ohps,
                tc.tile_pool(name="mm2_ps", bufs=1, space="PSUM") as mm2ps,
            ):
                for bi in range(NB):
                    bs = BSIZES[bi]
                    c0 = 4 * bi
                    nb = bs * 128

                    # gather phrase rows for this block's pairs
                    phrg = bp.tile([128, 4, D], F32R, name="phrg")
                    for j in range(bs):
                        nc.gpsimd.indirect_dma_start(
                            out=phrg[:, j, :], out_offset=None, in_=phr[:, :],
                            in_offset=bass.IndirectOffsetOnAxis(
                                ap=idx_sb[:, c0 + j:c0 + j + 1], axis=0))

                    # PE transpose to T layout [in, pairs]
                    phrT = bp.tile([128, NIC, 4, 128], F32R, name="phrT")
                    for ic in range(NIC):
                        for j in range(bs):
                            pst = trps.tile([128, 128], F32R, space="PSUM", name="pst")
                            nc.tensor.transpose(
                                pst, phrg[:, j, ic * 128:(ic + 1) * 128], ident_sb)
                            if (ic + j) % 3 == 0:
                                nc.scalar.activation(phrT[:, ic, j, :], pst, AF.Copy)
                            else:
                                nc.vector.tensor_copy(phrT[:, ic, j, :], pst)

                    # one-hot matrices for sub/obj of this block
                    ohs = bp.tile([NI, 4, 128], F32R, name="ohs")
                    oho = bp.tile([NI, 4, 128], F32R, name="oho")
                    ohsf = bp.tile([NI, 4, 128], F32, name="ohsf")
                    ohof = bp.tile([NI, 4, 128], F32, name="ohof")
                    for oh, ohf, srcd in ((ohs, ohsf, subf), (oho, ohof, objf)):
                        srow = bp.tile([1, 512], F32R, name="srow")
                        nc.sync.dma_start(out=srow[:, :nb],
                                          in_=srcd[0:1, c0 * 128:(c0 + bs) * 128])
                        ps_oh = ohps.tile([NI, 4, 128], F32, space="PSUM", name="ps_oh")
                        nc.tensor.matmul(
                            out=ps_oh[:, :bs, :].rearrange("p b x -> p (b x)"),
                            lhsT=ones1_sb,
                            rhs=srow[0:1, :nb],
                            start=True, stop=True)
                        rep = bp.tile([NI, 4, 128], F32, name="rep")
                        nc.scalar.activation(rep[:, :bs, :], ps_oh[:, :bs, :], AF.Copy)
                        nc.vector.tensor_tensor(
                            out=oh[:, :bs, :],
                            in0=iota_sb.to_broadcast([NI, bs, 128]),
                            in1=rep[:, :bs, :], op=ALU.is_equal)
                        nc.vector.tensor_tensor(
                            out=ohf[:, :bs, :],
                            in0=iota_sb.bitcast(F32).to_broadcast([NI, bs, 128]),
                            in1=rep[:, :bs, :], op=ALU.is_equal)

                    # mm1: pre_h_T accumulation + bias + leaky relu
                    hT = bp.tile([128, NHC, 4, 128], F32R, name="hT")
                    for hc in range(NHC):
                        ps_h = mm1ps.tile([128, 512], F32, space="PSUM", name="ps_h")
                        for ic in range(NIC):
                            nc.tensor.matmul(
                                out=ps_h[:, :nb],
                                lhsT=w1c_sb[:, ic, hc * 128:(hc + 1) * 128],
                                rhs=phrT[:, ic, :bs, :].rearrange("p b x -> p (b x)"),
                                start=(ic == 0), stop=False)
                        nc.tensor.matmul(
                            out=ps_h[:, :nb],
                            lhsT=A_r[:, hc * 128:(hc + 1) * 128],
                            rhs=ohs[:, :bs, :].rearrange("p b x -> p (b x)"),
                            start=False, stop=False)
                        nc.tensor.matmul(
                            out=ps_h[:, :nb],
                            lhsT=B_r[:, hc * 128:(hc + 1) * 128],
                            rhs=oho[:, :bs, :].rearrange("p b x -> p (b x)"),
                            start=False, stop=True)
                        nc.scalar.activation(
                            hT[:, hc, :bs, :].rearrange("p b x -> p (b x)"),
                            ps_h[:, :nb], AF.Lrelu,
                            bias=b1c_sb[:, hc:hc + 1], alpha=0.01)

                    # mm2: logits per column chunk
                    ps_lg = mm2ps.tile([128, 4, C + 1], F32, space="PSUM", name="ps_lg")
                    for j in range(bs):
                        for hc in range(NHC):
                            nc.tensor.matmul(out=ps_lg[:, j, :],
                                             lhsT=hT[:, hc, j, :],
                                             rhs=w2_sb[:, hc, :],
                                             start=(hc == 0), stop=(hc == NHC - 1))
                    lg = bp.tile([128, 4, C], F32, name="lg")
                    nc.vector.tensor_tensor(
                        out=lg[:, :bs, :], in0=ps_lg[:, :bs, :C],
                        in1=b2r_sb.unsqueeze(1).to_broadcast([128, bs, C]),
                        op=ALU.add)
                    nc.sync.dma_start(
                        out=o_logits[:, :].rearrange("(c p) k -> p c k", p=128)[:, c0:c0 + bs, :],
                        in_=lg[:, :bs, :])

                    # softmax (no max-subtraction; logits are O(1))
                    e = bp.tile([128, 4, C], F32, name="e")
                    se = bp.tile([128, 4], F32, name="se")
                    for j in range(bs):
                        nc.scalar.activation(e[:, j, :], lg[:, j, :], AF.Exp,
                                             accum_out=se[:, j:j + 1])
                    re = bp.tile([128, 4], F32, name="re")
                    nc.vector.reciprocal(re[:, :bs], se[:, :bs])
                    pb = bp.tile([128, 4, C], F32, name="pb")
                    for j in range(bs):
                        nc.scalar.activation(pb[:, j, :], e[:, j, :], AF.Copy,
                                             scale=re[:, j:j + 1])
                    nc.sync.dma_start(
                        out=o_probs[:, :].rearrange("(c p) k -> p c k", p=128)[:, c0:c0 + bs, :],
                        in_=pb[:, :bs, :])

                    # per-pair class max (class 0 excluded) + ss matmuls
                    nc.vector.tensor_reduce(ppmat[:, c0:c0 + bs],
                                            pb[:, :bs, 1:C],
                                            axis=mybir.AxisListType.X,
                                            op=ALU.max)
                    for j in range(bs):
                        nc.tensor.matmul(out=ss_sub[:, c0 + j:c0 + j + 1],
                                         lhsT=ohsf[:, j, :], rhs=scores_sb,
                                         start=True, stop=True)
                        nc.tensor.matmul(out=ss_obj[:, c0 + j:c0 + j + 1],
                                         lhsT=ohof[:, j, :], rhs=scores_sb,
                                         start=True, stop=True)

            if _STAGE[0] < 1.5:
                return nc
            # ---- overall score + padding mask ----
            with tc.tile_pool(name="topk_sb", bufs=1) as tk:
                sssb = tk.tile([128, COLS], F32)
                nc.scalar.activation(sssb, ss_sub, AF.Copy)
                ssp = tk.tile([128, COLS], F32)
                nc.vector.tensor_tensor(out=ssp, in0=sssb, in1=ss_obj,
                                        op=ALU.mult)
                nc.sync.dma_start(
                    out=ssp_d[:, :].rearrange("(c p) o -> p (c o)", p=128),
                    in_=ssp)
                ov0 = tk.tile([128, COLS], F32)
                nc.vector.tensor_tensor(out=ov0, in0=ppmat, in1=ssp, op=ALU.mult)
                overall = tk.tile([128, COLS], F32)
                nc.vector.tensor_tensor(out=overall, in0=ov0, in1=pmask_sb,
                                        op=ALU.min)

                # candidates: per-partition top-8 + global pair index
                m8 = tk.tile([128, 8], F32)
                i8 = tk.tile([128, 8], U32)
                nc.vector.max(out=m8, in_=overall)
                nc.vector.max_index(out=i8, in_max=m8, in_values=overall)
                gfa = tk.tile([128, 8], F32)
                nc.vector.tensor_copy(gfa, i8)
                gf = tk.tile([128, 8], F32)
                nc.vector.tensor_scalar(gf, gfa, 128.0, ip78_sb[:, 0:1],
                                        op0=ALU.mult, op1=ALU.add)
                gint = tk.tile([128, 8], I32)
                nc.vector.tensor_copy(gint, gf)

                # --- exact fp32 recompute + re-ranking of the 1024 candidates ---
                css = tk.tile([128, 8], F32)
                cmeta = tk.tile([128, 8, 4], I32)
                for e in range(8):
                    nc.gpsimd.indirect_dma_start(
                        out=css[:, e:e + 1], out_offset=None, in_=ssp_d[:, :],
                        in_offset=bass.IndirectOffsetOnAxis(
                            ap=gint[:, e:e + 1], axis=0))
                    nc.gpsimd.indirect_dma_start(
                        out=cmeta[:, e, :], out_offset=None, in_=cat32[:, :],
                        in_offset=bass.IndirectOffsetOnAxis(
                            ap=gint[:, e:e + 1], axis=0))
                cpi = tk.tile([128, 8], I32)
                nc.vector.tensor_copy(cpi, cmeta[:, :, 0])

                # sub/obj values of candidates as flat [1,1024] rows
                csubF = tk.tile([128, 8], F32)
                nc.vector.tensor_copy(csubF, cmeta[:, :, 1])
                cobjF = tk.tile([128, 8], F32)
                nc.vector.tensor_copy(cobjF, cmeta[:, :, 2])
                # transpose to [8,128] so the flat row is in e*128+p order
                with tc.tile_pool(name="ctr_ps", bufs=1, space="PSUM") as ctrp:
                    csub_row = tk.tile([1, 1024], F32)
                    cobj_row = tk.tile([1, 1024], F32)
                    for colF, row in ((csubF, csub_row), (cobjF, cobj_row)):
                        psct = ctrp.tile([8, 128], F32, space="PSUM", name="psct")
                        nc.tensor.transpose(psct, colF, ident_sb.bitcast(F32))
                        ctT = tk.tile([8, 128], F32, name="ctT", tag="ctT")
                        nc.vector.tensor_copy(ctT, psct)
                        nc.sync.dma_start(
                            out=row.rearrange("o (q r) -> o q r", q=8), in_=ctT)

                # fp32 W1c reload (raw bytes view of the f32r param)
                w1cf_sb = tk.tile([128, NIC, H], F32)
                nc.sync.dma_start(
                    out=w1cf_sb,
                    in_=w1c[:, :].bitcast(F32).rearrange("(ic p) h -> p ic h", p=128))

                clg = tk.tile([128, 8, C], F32)
                with (
                    tc.tile_pool(name="ex_sb", bufs=1) as xb,
                    tc.tile_pool(name="ex_tr", bufs=2, space="PSUM") as xtr,
                    tc.tile_pool(name="ex_mm1", bufs=2, space="PSUM") as xmm1,
                    tc.tile_pool(name="ex_oh", bufs=1, space="PSUM") as xoh,
                    tc.tile_pool(name="ex_mm2", bufs=1, space="PSUM") as xmm2,
                ):
                    for cb in range(2):
                        er = range(4 * cb, 4 * cb + 4)
                        # gather candidate phrase rows (exact fp32 bytes)
                        phrgc = xb.tile([128, 4, D], F32, name="phrgc", bufs=2)
                        for k, e in enumerate(er):
                            nc.gpsimd.indirect_dma_start(
                                out=phrgc[:, k, :], out_offset=None,
                                in_=phr[:, :].bitcast(F32),
                                in_offset=bass.IndirectOffsetOnAxis(
                                    ap=cpi[:, e:e + 1], axis=0))
                        phrTc = xb.tile([128, NIC, 4, 128], F32, name="phrTc")
                        for ic in range(NIC):
                            for k in range(4):
                                pstx = xtr.tile([128, 128], F32, space="PSUM",
                                                name="pstx")
                                nc.tensor.transpose(
                                    pstx, phrgc[:, k, ic * 128:(ic + 1) * 128],
                                    ident_sb.bitcast(F32))
                                if (ic + k) % 2 == 0:
                                    nc.scalar.activation(phrTc[:, ic, k, :], pstx,
                                                         AF.Copy)
                                else:
                                    nc.vector.tensor_copy(phrTc[:, ic, k, :], pstx)
                        # candidate one-hots (fp32)
                        ohcs = xb.tile([NI, 4, 128], F32, name="ohcs")
                        ohco = xb.tile([NI, 4, 128], F32, name="ohco")
                        for ohc, rowt in ((ohcs, csub_row), (ohco, cobj_row)):
                            ps_ohc = xoh.tile([NI, 4, 128], F32, space="PSUM",
                                              name="ps_ohc")
                            nc.tensor.matmul(
                                out=ps_ohc.rearrange("p b x -> p (b x)"),
                                lhsT=ones1f_sb,
                                rhs=rowt[0:1, 512 * cb:512 * (cb + 1)],
                                start=True, stop=True)
                            repc = xb.tile([NI, 4, 128], F32, name="repc")
                            nc.scalar.activation(repc, ps_ohc, AF.Copy)
                            nc.vector.tensor_tensor(
                                out=ohc,
                                in0=iota_sb.bitcast(F32).to_broadcast([NI, 4, 128]),
                                in1=repc, op=ALU.is_equal)
                        # exact mm1 + leaky
                        hTc = xb.tile([128, NHC, 4, 128], F32, name="hTc")
                        for hc in range(NHC):
                            psx = xmm1.tile([128, 512], F32, space="PSUM", name="psx")
                            for ic in range(NIC):
                                nc.tensor.matmul(
                                    out=psx,
                                    lhsT=w1cf_sb[:, ic, hc * 128:(hc + 1) * 128],
                                    rhs=phrTc[:, ic, :, :].rearrange("p b x -> p (b x)"),
                                    start=(ic == 0), stop=False)
                            nc.tensor.matmul(
                                out=psx, lhsT=A_sb[:, hc * 128:(hc + 1) * 128],
                                rhs=ohcs.rearrange("p b x -> p (b x)"),
                                start=False, stop=False)
                            nc.tensor.matmul(
                                out=psx, lhsT=B_sb[:, hc * 128:(hc + 1) * 128],
                                rhs=ohco.rearrange("p b x -> p (b x)"),
                                start=False, stop=True)
                            nc.scalar.activation(
                                hTc[:, hc, :, :].rearrange("p b x -> p (b x)"),
                                psx, AF.Lrelu, bias=b1c_sb[:, hc:hc + 1],
                                alpha=0.01)
                        # exact mm2 + b2
                        ps_lx = xmm2.tile([128, 4, C], F32, space="PSUM", name="ps_lx")
                        for k in range(4):
                            for hc in range(NHC):
                                nc.tensor.matmul(out=ps_lx[:, k, :],
                                                 lhsT=hTc[:, hc, k, :],
                                                 rhs=w2f_sb[:, hc, :C],
                                                 start=(hc == 0),
                                                 stop=(hc == NHC - 1))
                        nc.vector.tensor_tensor(
                            out=clg[:, 4 * cb:4 * cb + 4, :], in0=ps_lx,
                            in1=b2r_sb.unsqueeze(1).to_broadcast([128, 4, C]),
                            op=ALU.add)

                # stable softmax pieces with polynomial exp (~1e-7 rel)
                lmax = tk.tile([128, 8], F32)
                nc.vector.tensor_reduce(lmax, clg, axis=mybir.AxisListType.X,
                                        op=ALU.max)
                x = tk.tile([128, 8, C], F32)
                nc.vector.tensor_tensor(
                    out=x, in0=clg,
                    in1=lmax.unsqueeze(2).to_broadcast([128, 8, C]),
                    op=ALU.subtract)
                LOG2E, LN2 = 1.4426950408889634, 0.6931471805599453
                t = tk.tile([128, 8, C], F32)
                nc.vector.tensor_scalar(t, x, LOG2E, None, op0=ALU.mult)
                ki = tk.tile([128, 8, C], I32)
                nc.vector.tensor_copy(ki, t)
                kf = tk.tile([128, 8, C], F32)
                nc.vector.tensor_copy(kf, ki)
                u = tk.tile([128, 8, C], F32)
                nc.vector.tensor_scalar(u, kf, LN2, None, op0=ALU.mult)
                r = tk.tile([128, 8, C], F32)
                nc.vector.tensor_tensor(out=r, in0=x, in1=u, op=ALU.subtract)
                ke0 = tk.tile([128, 8, C], I32)
                nc.vector.tensor_scalar(ke0, ki, 127, None, op0=ALU.add)
                ke = tk.tile([128, 8, C], I32)
                nc.vector.tensor_scalar(ke, ke0, 23, None,
                                        op0=ALU.arith_shift_left)
                # Horner for e^r, r in (-ln2, ln2)
                import math
                coef = [1.0 / math.factorial(n) for n in range(10)]
                pol = tk.tile([128, 8, C], F32)
                nc.vector.tensor_scalar(pol, r, coef[9], coef[8],
                                        op0=ALU.mult, op1=ALU.add)
                for n in range(7, -1, -1):
                    pm = tk.tile([128, 8, C], F32, name=f"pm{n}", tag="pm")
                    nc.vector.tensor_tensor(out=pm, in0=pol, in1=r, op=ALU.mult)
                    pol = tk.tile([128, 8, C], F32, name=f"po{n}", tag="po")
                    nc.vector.tensor_scalar(pol, pm, coef[n], None, op0=ALU.add)
                eall = tk.tile([128, 8, C], F32)
                nc.vector.tensor_tensor(out=eall, in0=pol,
                                        in1=ke.bitcast(F32), op=ALU.mult)
                sexp = tk.tile([128, 8], F32)
                nc.vector.tensor_reduce(sexp, eall, axis=mybir.AxisListType.X,
                                        op=ALU.add)
                # reciprocal + one Newton step
                r0 = tk.tile([128, 8], F32)
                nc.vector.reciprocal(r0, sexp)
                t2 = tk.tile([128, 8], F32)
                nc.vector.tensor_tensor(out=t2, in0=sexp, in1=r0, op=ALU.mult)
                u2 = tk.tile([128, 8], F32)
                nc.vector.tensor_scalar(u2, t2, -1.0, 2.0,
                                        op0=ALU.mult, op1=ALU.add)
                r1 = tk.tile([128, 8], F32)
                nc.vector.tensor_tensor(out=r1, in0=r0, in1=u2, op=ALU.mult)
                # class-0-excluded max + argmax
                emax1 = tk.tile([128, 8], F32)
                labs = tk.tile([128, 8], I32)
                for e in range(8):
                    cm8 = tk.tile([128, 8], F32, name=f"cm8_{e}", tag="cm8")
                    ci8 = tk.tile([128, 8], U32, name=f"ci8_{e}", tag="ci8")
                    nc.vector.max(out=cm8, in_=eall[:, e, 1:C])
                    nc.vector.max_index(out=ci8, in_max=cm8,
                                        in_values=eall[:, e, 1:C])
                    nc.vector.tensor_copy(emax1[:, e:e + 1], cm8[:, 0:1])
                    nc.vector.tensor_scalar(labs[:, e:e + 1], ci8[:, 0:1], 1,
                                            None, op0=ALU.add)
                pred = tk.tile([128, 8], F32)
                nc.vector.tensor_tensor(out=pred, in0=emax1, in1=r1, op=ALU.mult)
                novr = tk.tile([128, 8], F32)
                nc.vector.tensor_tensor(out=novr, in0=pred, in1=css, op=ALU.mult)

                # stash candidate pred/label by candidate position j=p*8+e
                nc.sync.dma_start(
                    out=candp_d[:, :].rearrange("(p e) o -> p (e o)", e=8),
                    in_=pred)
                nc.sync.dma_start(
                    out=candl_d[:, :].rearrange("(p e) o -> p (e o)", e=8),
                    in_=labs)

                if _STAGE[0] < 2.5:
                    return nc
                rv = tk.tile([1, 1024], F32)
                nc.sync.dma_start(out=rv.rearrange("o (p e) -> o p e", p=128),
                                  in_=novr)
                rg = tk.tile([1, 1024], F32)
                nc.sync.dma_start(out=rg.rearrange("o (p e) -> o p e", p=128), in_=gf)
                nc.sync.dma_start(out=gx_d[:, :].rearrange("p o -> o p"), in_=rg)

                # 13 global extraction rounds -> top-104 sorted
                tvals = tk.tile([1, NROUND * 8], F32)
                tpos = tk.tile([1, NROUND * 8], F32)
                for k in range(NROUND):
                    rm8 = tk.tile([1, 8], F32, name="rm8", tag="rm8")
                    ri8 = tk.tile([1, 8], U32, name="ri8", tag="ri8")
                    nc.vector.max(out=rm8, in_=rv)
                    nc.vector.max_index(out=ri8, in_max=rm8, in_values=rv)
                    nc.vector.match_replace(out=rv, in_to_replace=rm8,
                                            in_values=rv, imm_value=-1e30)
                    nc.vector.tensor_copy(tvals[:, 8 * k:8 * k + 8], rm8)
                    nc.vector.tensor_copy(tpos[:, 8 * k:8 * k + 8], ri8)

                # positions -> column vector -> int
                tposc = tk.tile([NROUND * 8, 1], F32)
                nc.sync.dma_start(
                    out=tposc, in_=tpos.rearrange("o (p e) -> o p e", p=NROUND * 8))
                tposi = tk.tile([NROUND * 8, 1], I32)
                nc.vector.tensor_copy(tposi, tposc)

                # gather winner pair idx / pred prob / label by position
                gxc = tk.tile([NROUND * 8, 1], F32)
                nc.gpsimd.indirect_dma_start(
                    out=gxc, out_offset=None, in_=gx_d[:, :],
                    in_offset=bass.IndirectOffsetOnAxis(ap=tposi[:, :1], axis=0))
                gxi = tk.tile([NROUND * 8, 1], I32)
                nc.vector.tensor_copy(gxi, gxc)
                tprobc = tk.tile([NROUND * 8, 1], F32)
                nc.gpsimd.indirect_dma_start(
                    out=tprobc, out_offset=None, in_=candp_d[:, :],
                    in_offset=bass.IndirectOffsetOnAxis(ap=tposi[:, :1], axis=0))
                tlabc = tk.tile([NROUND * 8, 1], I32)
                nc.gpsimd.indirect_dma_start(
                    out=tlabc, out_offset=None, in_=candl_d[:, :],
                    in_offset=bass.IndirectOffsetOnAxis(ap=tposi[:, :1], axis=0))

                # outputs
                nc.sync.dma_start(out=o_tval[:, :], in_=tvals[:, :TOPK])
                nc.sync.dma_start(
                    out=o_tidx[:, :].rearrange("o (p e) -> o p e", p=TOPK),
                    in_=gxi[:TOPK, :])
                nc.sync.dma_start(
                    out=o_tlab[:, :].rearrange("o (p e) -> o p e", p=TOPK),
                    in_=tlabc[:TOPK, :])
                nc.sync.dma_start(
                    out=o_tprob[:, :].rearrange("o (p e) -> o p e", p=TOPK),
                    in_=tprobc[:TOPK, :])

    _fix_multiwait(nc)
    return nc


def _fix_multiwait(nc, keep=1):
    """walrus (this snapshot) rejects instructions whose sync_info carries
    more waits than the ISA CTRL struct holds (the Tile tail Drain gets one
    wait per outstanding semaphore). Spill excess waits onto standalone NOPs
    inserted immediately before the instruction — NX executes them in program
    order, so semantics are identical."""
    import concourse.mybir as mybir
    ctr = [0]
    for f in nc.m.functions:
        for bb in f.blocks:
            insts = bb.instructions
            i = 0
            while i < len(insts):
                ins = insts[i]
                si = ins.sync_info
                if si is not None and len(si.on_wait) > keep:
                    waits = list(si.on_wait)
                    spill, rest = waits[:-keep], waits[-keep:]
                    for w in spill:
                        ctr[0] += 1
                        nop = mybir.InstNoOp(
                            name=f"I-waitspill-{ctr[0]}", ins=[], outs=[])
                        nop.engine = ins.engine
                        nop.sync_info = mybir.SyncInfo(on_wait=[w], on_update=[])
                        insts.insert(i, nop)
                        i += 1
                    ins.sync_info = mybir.SyncInfo(
                        on_wait=rest, on_update=list(si.on_update))
                i += 1


def _host_prep(inputs):
    """Per-core input maps. All host work is layout/marshalling only."""
    inst = np.asarray(inputs["instance_features"], np.float32)   # [B,100,1024]
    phr = np.asarray(inputs["phrase_features"], np.float32)      # [B,9900,1024]
    scores = np.asarray(inputs["scores"], np.float32)            # [B,100]
    W1 = np.asarray(inputs["W1"], np.float32)                    # [3072,1024]
    b1 = np.asarray(inputs["b1"], np.float32)
    W2 = np.asarray(inputs["W2"], np.float32)
    b2 = np.asarray(inputs["b2"], np.float32)
    conn = np.asarray(inputs["connect_arr"])                     # [B,2,9900]
    pidx = np.asarray(inputs["phrase_idx"])                      # [B,9900]

    w2p = np.concatenate([W2, np.zeros((H, 1), np.float32)], axis=1)
    w1a = np.ascontiguousarray(W1[:D])
    w1b = np.ascontiguousarray(W1[D:2 * D])
    w1c = np.ascontiguousarray(W1[2 * D:])
    b1c = np.ascontiguousarray(b1.reshape(NHC, 128).T)           # [128,8]
    b2r = np.ascontiguousarray(np.broadcast_to(b2, (128, C)))
    iota = np.arange(NI, dtype=np.float32).reshape(NI, 1)
    ident = np.eye(128, dtype=np.float32)
    ip78 = np.arange(128, dtype=np.float32).reshape(128, 1)
    pmask = np.ascontiguousarray(
        np.where(np.arange(PADP) < P, 1e30, -1e30).astype(np.float32)
        .reshape(COLS, 128).T)

    def pad_pair(a, fill=0):
        out = np.full(PADP, fill, a.dtype)
        out[:P] = a
        return out

    maps = []
    for b in range(B):
        sub_i = conn[b, 0].astype(np.int64)
        obj_i = conn[b, 1].astype(np.int64)
        pi = pidx[b].astype(np.int64)
        # partition-major order: pair = p*78 + c  ->  array[p, c]
        subf = pad_pair(sub_i.astype(np.float32)).reshape(1, PADP)
        objf = pad_pair(obj_i.astype(np.float32)).reshape(1, PADP)
        idx32 = np.ascontiguousarray(pad_pair(pi.astype(np.int32)).reshape(COLS, 128).T)
        maps.append({
            "phr": np.ascontiguousarray(phr[b]),
            "w1a": w1a, "w1b": w1b, "w1c": w1c,
            "instT": np.ascontiguousarray(inst[b].T),
            "w2": w2p, "b1c": b1c, "b2r": b2r,
            "scoresc": np.ascontiguousarray(scores[b].reshape(NI, 1)),
            "subf": np.ascontiguousarray(subf),
            "objf": np.ascontiguousarray(objf),
            "idx32": idx32,
            "iota100": iota, "identf": ident, "ip78": ip78,
            "padmask": pmask,
            "cat32": np.ascontiguousarray(np.stack(
                [pi, sub_i, obj_i, np.zeros(P, np.int64)], axis=1).astype(np.int32)),
        })
    return maps


def kernel(**inputs):
    from concourse.bass_utils import run_bass_kernel_spmd

    if "nc" not in _cache:
        _cache["nc"] = _build()
    nc = _cache["nc"]

    maps = _host_prep(inputs)
    res = run_bass_kernel_spmd(nc, maps, list(range(B))).results

    logits = np.stack([res[b]["o_logits"][:P] for b in range(B)])
    probs = np.stack([res[b]["o_probs"][:P] for b in range(B)])
    tlab = np.stack([res[b]["o_tlab"][0] for b in range(B)]).astype(np.int32)
    tprob = np.stack([res[b]["o_tprob"][0] for b in range(B)])
    tval = np.stack([res[b]["o_tval"][0] for b in range(B)])
    tidx = np.stack([res[b]["o_tidx"][0] for b in range(B)]).astype(np.int32)
    return logits, probs, tlab, tprob, tval, tidx
